# revision 1
# baseline (speedup 1.0000x reference)
"""GCN 2-layer kernel for trn2 (8 NeuronCores, SPMD).

Sharding: nodes dst-sharded across 8 cores (12500 each, padded to 12544).
Each core owns the edges whose dst lands in its shard (plus self-loops),
packed into 128-edge chunks, 37 chunks per 128-dst block (static layout).
Layer tables (g = dinv * (x@W+b)) are built per-shard on the TensorEngine
in bf16 and AllGathered. Messages are fetched with dma_gather (L1: 8-row
bf16 packs, 256B elems; L2: 4-row bf16 packs, 512B elems; int16 pack
indices), lane-MASKED on DVE (one is_eq + one multiply per group), and
scatter-added per dst-block with wide one-hot matmuls on the TensorEngine
accumulating lane-resolved columns in PSUM; a cheap per-block lane-sum
collapses the pack lanes. log_softmax runs on-device; host only shards,
permutes, and concatenates.
"""
import sys
import numpy as np

if "/opt/trn_rl_repo" not in sys.path:
    sys.path.insert(0, "/opt/trn_rl_repo")

import ml_dtypes

BF16 = ml_dtypes.bfloat16

N = 100000
NCORES = 8
SHARD = 12500
SHARD_PAD = 12544          # 128 * 98
NBLK = 98                  # dst blocks of 128 per core
BPC = 34                   # chunks per block (128 edges each)
NCHUNK = NBLK * BPC        # 3626 PE chunks
NI = 1024                  # edges per dma_gather instruction
CPG = NI // 128            # 8 chunks per gather group
NGI = (NCHUNK + CPG - 1) // CPG   # 454 gather instructions per layer
NCHUNK_G = NGI * CPG       # 3632 chunk slots incl. trailing pads
NFEAT = 512
NHID = 16
NCLASS = 40
CPAD = 48                  # padded class dim
L2W = 64                   # layer-2 table row width (CPAD + pad)
WCOL = NI // 16            # gidx columns per gather instruction (64)

_PROGRAM = None
_SLOT_OF = None


def _build_program():
    import concourse.bacc as bacc
    import concourse.mybir as mybir
    import concourse.tile as tile

    f32 = mybir.dt.float32
    bf16 = mybir.dt.bfloat16
    i16 = mybir.dt.int16
    Alu = mybir.AluOpType
    Act = mybir.ActivationFunctionType
    Axis = mybir.AxisListType

    nc = bacc.Bacc("TRN2", target_bir_lowering=False, debug=False,
                   num_devices=NCORES, num_swdge_queues=4)

    xT = nc.dram_tensor("xT", [NFEAT, SHARD_PAD], bf16,
                        kind="ExternalInput")
    rp0 = nc.dram_tensor("rp0", [128, NBLK], f32, kind="ExternalInput")
    rp1 = nc.dram_tensor("rp1", [128, NBLK], f32, kind="ExternalInput")
    gidx1 = nc.dram_tensor("gidx1", [128, NGI * WCOL], i16,
                           kind="ExternalInput")
    gidx2 = nc.dram_tensor("gidx2", [128, NGI * WCOL], i16,
                           kind="ExternalInput")
    lane1 = nc.dram_tensor("lane1", [128, NCHUNK_G], bf16,
                           kind="ExternalInput")
    lane2 = nc.dram_tensor("lane2", [128, NCHUNK_G], bf16,
                           kind="ExternalInput")
    dstl = nc.dram_tensor("dstl", [128, NCHUNK_G], bf16,
                          kind="ExternalInput")
    liota1 = nc.dram_tensor("liota1", [128, CPG * 128], bf16,
                            kind="ExternalInput")
    liota2 = nc.dram_tensor("liota2", [128, CPG * 256], bf16,
                            kind="ExternalInput")
    iota = nc.dram_tensor("iota", [128, CPG * 128], bf16,
                          kind="ExternalInput")
    w1 = nc.dram_tensor("w1", [NFEAT, NHID], bf16,
                        kind="ExternalInput")
    b1 = nc.dram_tensor("b1", [1, NHID], f32, kind="ExternalInput")
    w2 = nc.dram_tensor("w2", [NHID, CPAD], f32, kind="ExternalInput")
    b2 = nc.dram_tensor("b2", [1, CPAD], f32, kind="ExternalInput")
    ident = nc.dram_tensor("ident", [128, 128], f32, kind="ExternalInput")
    padmask = nc.dram_tensor("padmask", [128, 1], f32, kind="ExternalInput")
    out = nc.dram_tensor("out", [SHARD_PAD, NCLASS], f32,
                         kind="ExternalOutput")

    with tile.TileContext(nc) as tc:
        with (
            tc.tile_pool(name="const", bufs=1) as constp,
            tc.tile_pool(name="stream", bufs=4) as streamp,
            tc.tile_pool(name="gat", bufs=6) as gatp,
            tc.tile_pool(name="work", bufs=4) as workp,
            tc.tile_pool(name="fin", bufs=3) as finp,
            tc.tile_pool(name="psum", bufs=2, space="PSUM") as psump,
            tc.tile_pool(name="dram", bufs=1, space="DRAM") as dramp,
        ):
            # ---- constants ----
            iota_t = constp.tile([128, CPG * 128], bf16)
            nc.sync.dma_start(iota_t[:], iota[:])
            li1_t = constp.tile([128, CPG * 128], bf16)
            nc.sync.dma_start(li1_t[:], liota1[:])
            li2_t = constp.tile([128, CPG * 256], bf16)
            nc.sync.dma_start(li2_t[:], liota2[:])
            lane1_t = constp.tile([128, NCHUNK_G], bf16)
            nc.sync.dma_start(lane1_t[:], lane1[:])
            lane2_t = constp.tile([128, NCHUNK_G], bf16)
            nc.sync.dma_start(lane2_t[:], lane2[:])
            dstl_t = constp.tile([128, NCHUNK_G], bf16)
            nc.sync.dma_start(dstl_t[:], dstl[:])
            ident_t = constp.tile([128, 128], f32)
            nc.sync.dma_start(ident_t[:], ident[:])
            w1_t = constp.tile([128, NFEAT // 128, NHID], bf16)
            nc.sync.dma_start(w1_t[:],
                              w1[:].rearrange("(a k) h -> k a h", k=128))
            b1_t = constp.tile([1, NHID], f32)
            nc.sync.dma_start(b1_t[:], b1[:])
            w2_t = constp.tile([NHID, CPAD], f32)
            nc.sync.dma_start(w2_t[:], w2[:])
            b2_t = constp.tile([1, CPAD], f32)
            nc.sync.dma_start(b2_t[:], b2[:])
            ones_t = constp.tile([1, 128], f32)
            nc.vector.memset(ones_t[:], 1.0)
            ones_b = constp.tile([1, 128], bf16)
            nc.vector.memset(ones_b[:], 1.0)
            b1b_t = constp.tile([1, NHID], bf16)
            nc.vector.tensor_copy(out=b1b_t[:], in_=b1_t[:])

            # ---- dinv[p, c] for node 128c+p: rsqrt(deg + 1) ----
            dinv_t = constp.tile([128, NBLK], f32)
            rp0_t = workp.tile([128, NBLK], f32, tag="rp")
            rp1_t = workp.tile([128, NBLK], f32, tag="rp")
            nc.sync.dma_start(rp0_t[:], rp0[:])
            nc.sync.dma_start(rp1_t[:], rp1[:])
            deg_t = workp.tile([128, NBLK], f32, tag="deg")
            nc.vector.tensor_tensor(out=deg_t[:], in0=rp1_t[:], in1=rp0_t[:],
                                    op=Alu.subtract)
            sqd_t = workp.tile([128, NBLK], f32, tag="deg")
            nc.scalar.activation(out=sqd_t[:], in_=deg_t[:], func=Act.Sqrt,
                                 bias=1.0)
            nc.vector.reciprocal(out=dinv_t[:], in_=sqd_t[:])
            pm_t = constp.tile([128, 1], f32)
            nc.sync.dma_start(pm_t[:], padmask[:])
            dinv97_t = constp.tile([128, 1], f32)
            nc.vector.tensor_tensor(out=dinv97_t[:],
                                    in0=dinv_t[:, NBLK - 1:NBLK],
                                    in1=pm_t[:], op=Alu.mult)

            def dcol_of(b):
                return dinv97_t[:] if b == NBLK - 1 else dinv_t[:, b:b + 1]

            # ---- DRAM tables (bf16) ----
            t1shard = dramp.tile([SHARD_PAD, NHID], bf16)
            t1full = dramp.tile([SHARD_PAD * NCORES, NHID], bf16)
            t2shard = dramp.tile([SHARD_PAD, L2W], bf16)
            t2full = dramp.tile([SHARD_PAD * NCORES, L2W], bf16)

            # ---- phase 1: table1 rows = dinv * (x @ W1 + b1) ----
            for c in range(NBLK):
                ph = psump.tile([128, NHID], f32, tag="ph1")
                xk = streamp.tile([128, NFEAT // 128, 128], bf16, tag="xk")
                nc.sync.dma_start(
                    xk[:],
                    xT[:, c * 128:(c + 1) * 128].rearrange(
                        "(k p) c -> p k c", p=128))
                for kk in range(NFEAT // 128):
                    nc.tensor.matmul(out=ph[:], lhsT=xk[:, kk, :],
                                     rhs=w1_t[:, kk, :],
                                     start=(kk == 0), stop=False)
                nc.tensor.matmul(out=ph[:], lhsT=ones_b[:], rhs=b1b_t[:],
                                 start=False, stop=True)
                hs = workp.tile([128, NHID], bf16, tag="hs1")
                nc.scalar.activation(out=hs[:], in_=ph[:], func=Act.Copy,
                                     scale=dcol_of(c))
                nc.sync.dma_start(t1shard[c * 128:(c + 1) * 128, :], hs[:])

            nc.gpsimd.collective_compute(
                "AllGather", Alu.bypass,
                ins=[t1shard[:].opt()], outs=[t1full[:].opt()],
                replica_groups=[list(range(NCORES))],
            )
            t1packs = t1full[:].rearrange("(q r) h -> q (r h)", r=8)

            # ---- finishers ----
            def finish_block1(b, acc):
                # acc: [128, 128] psum f32, cols = 16*lane + feat
                cp = finp.tile([128, 128], f32, tag="cp1")
                nc.scalar.activation(out=cp[:], in_=acc[:, 0:128],
                                     func=Act.Copy)
                s1 = finp.tile([128, 64], f32, tag="s1a")
                nc.vector.tensor_tensor(out=s1[:], in0=cp[:, 0:64],
                                        in1=cp[:, 64:128], op=Alu.add)
                s2 = finp.tile([128, 32], f32, tag="s2a")
                nc.vector.tensor_tensor(out=s2[:], in0=s1[:, 0:32],
                                        in1=s1[:, 32:64], op=Alu.add)
                s3 = finp.tile([128, NHID], f32, tag="s3a")
                nc.vector.tensor_tensor(out=s3[:], in0=s2[:, 0:16],
                                        in1=s2[:, 16:32], op=Alu.add)
                a1 = finp.tile([128, NHID], f32, tag="a1")
                nc.scalar.activation(out=a1[:], in_=s3[:], func=Act.Relu,
                                     scale=dcol_of(b))
                pt = psump.tile([NHID, 128], f32, tag="ptr")
                nc.tensor.transpose(out=pt[:], in_=a1[:], identity=ident_t[:])
                a1T = finp.tile([NHID, 128], f32, tag="a1T")
                nc.vector.tensor_copy(out=a1T[:], in_=pt[:])
                ph2 = psump.tile([128, CPAD], f32, tag="ph2")
                nc.tensor.matmul(out=ph2[:], lhsT=a1T[:], rhs=w2_t[:],
                                 start=True, stop=False)
                nc.tensor.matmul(out=ph2[:], lhsT=ones_t[:], rhs=b2_t[:],
                                 start=False, stop=True)
                h2 = finp.tile([128, L2W], bf16, tag="h2")
                nc.vector.memset(h2[:, CPAD:], 0.0)
                nc.scalar.activation(out=h2[:, :CPAD], in_=ph2[:],
                                     func=Act.Copy, scale=dcol_of(b))
                nc.sync.dma_start(t2shard[b * 128:(b + 1) * 128, :], h2[:])

            def finish_block2(b, acc):
                # acc: [128, 256] psum f32, cols = 64*lane + feat
                cp = finp.tile([128, 256], f32, tag="cp2")
                nc.scalar.activation(out=cp[:], in_=acc[:], func=Act.Copy)
                s1 = finp.tile([128, 128], f32, tag="s1b")
                nc.vector.tensor_tensor(out=s1[:], in0=cp[:, 0:128],
                                        in1=cp[:, 128:256], op=Alu.add)
                s2 = finp.tile([128, L2W], f32, tag="s2b")
                nc.vector.tensor_tensor(out=s2[:], in0=s1[:, 0:64],
                                        in1=s1[:, 64:128], op=Alu.add)
                o2 = finp.tile([128, CPAD], f32, tag="o2")
                nc.scalar.activation(out=o2[:], in_=s2[:, :CPAD],
                                     func=Act.Copy, scale=dcol_of(b))
                rmax = finp.tile([128, 1], f32, tag="rmax")
                nc.vector.tensor_reduce(out=rmax[:], in_=o2[:, :NCLASS],
                                        axis=Axis.X, op=Alu.max)
                nrmax = finp.tile([128, 1], f32, tag="nrmax")
                nc.vector.tensor_scalar(out=nrmax[:], in0=rmax[:],
                                        scalar1=-1.0, scalar2=None,
                                        op0=Alu.mult)
                sh = finp.tile([128, NCLASS], f32, tag="sh")
                nc.scalar.activation(out=sh[:], in_=o2[:, :NCLASS],
                                     func=Act.Identity, bias=nrmax[:])
                ex = finp.tile([128, NCLASS], f32, tag="ex")
                nc.scalar.activation(out=ex[:], in_=sh[:], func=Act.Exp)
                rsum = finp.tile([128, 1], f32, tag="rsum")
                nc.vector.tensor_reduce(out=rsum[:], in_=ex[:],
                                        axis=Axis.X, op=Alu.add)
                rinv = finp.tile([128, 1], f32, tag="rinv")
                nc.vector.reciprocal(out=rinv[:], in_=rsum[:])
                nlsum = finp.tile([128, 1], f32, tag="nlsum")
                nc.scalar.activation(out=nlsum[:], in_=rinv[:], func=Act.Ln)
                res = finp.tile([128, NCLASS], f32, tag="res")
                nc.scalar.activation(out=res[:], in_=sh[:],
                                     func=Act.Identity, bias=nlsum[:])
                nc.sync.dma_start(out[b * 128:(b + 1) * 128, :], res[:])

            # ---- message-passing phase builder ----
            def mp_phase(gidx_ap, packs_ap, elem, lane_tt, li_t, finish,
                         tagsfx):
                acc_holder = [None]
                for gi in range(NGI):
                    cs = slice(gi * CPG, gi * CPG + CPG)
                    gx = gatp.tile([128, WCOL], i16, tag="gx" + tagsfx,
                                   bufs=16)
                    nc.sync.dma_start(gx[:],
                                      gidx_ap[:, gi * WCOL:(gi + 1) * WCOL])
                    gat = gatp.tile([128, CPG, elem], bf16,
                                    tag="gt" + tagsfx,
                                    bufs=10 if tagsfx == "1" else 8)
                    nc.gpsimd.dma_gather(gat[:], packs_ap, gx[:], NI, NI,
                                         elem, queue_num=gi % 4)
                    msk = workp.tile([128, CPG, elem], bf16,
                                     tag="mk" + tagsfx,
                                     bufs=6 if tagsfx == "1" else 5)
                    nc.vector.tensor_tensor(
                        out=msk[:],
                        in0=lane_tt[:, cs].to_broadcast([128, CPG, elem]),
                        in1=li_t[:].rearrange("p (c e) -> p c e", e=elem),
                        op=Alu.is_equal)
                    gm = workp.tile([128, CPG, elem], bf16,
                                    tag="gm" + tagsfx,
                                    bufs=6 if tagsfx == "1" else 5)
                    nc.vector.tensor_tensor(out=gm[:], in0=gat[:],
                                            in1=msk[:], op=Alu.mult)
                    onehot = gatp.tile([128, CPG, 128], bf16,
                                       tag="oh" + tagsfx,
                                       bufs=10 if tagsfx == "1" else 8)
                    nc.vector.tensor_tensor(
                        out=onehot[:],
                        in0=dstl_t[:, cs].to_broadcast([128, CPG, 128]),
                        in1=iota_t[:].rearrange("p (c e) -> p c e", e=128),
                        op=Alu.is_equal)
                    for j in range(CPG):
                        c = gi * CPG + j
                        if c >= NCHUNK:
                            continue
                        b, jj = divmod(c, BPC)
                        if jj == 0:
                            acc_holder[0] = psump.tile(
                                [128, 256], f32, tag="acc", name="acc_t")
                        for h in range(elem // 128):
                            # start=True clears has_written for the WHOLE
                            # psum bank — issue it only once per block; the
                            # h>0 region then overwrites via cleared bits.
                            nc.tensor.matmul(
                                out=acc_holder[0][:, h * 128:(h + 1) * 128],
                                lhsT=onehot[:, j, :],
                                rhs=gm[:, j, h * 128:(h + 1) * 128],
                                start=(jj == 0 and h == 0),
                                stop=(jj == BPC - 1))
                        if jj == BPC - 1:
                            finish(b, acc_holder[0])

            # ---- phase 2: layer 1 edges ----
            mp_phase(gidx1[:], t1packs, 128, lane1_t, li1_t,
                     finish_block1, "1")

            nc.gpsimd.collective_compute(
                "AllGather", Alu.bypass,
                ins=[t2shard[:].opt()], outs=[t2full[:].opt()],
                replica_groups=[list(range(NCORES))],
            )
            t2packs = t2full[:].rearrange("(q r) h -> q (r h)", r=4)

            # ---- phase 4: layer 2 edges ----
            mp_phase(gidx2[:], t2packs, 256, lane2_t, li2_t,
                     finish_block2, "2")

    nc.compile()
    return nc


def _host_prep(x, edge_index, W1, b1, W2, b2):
    src = np.asarray(edge_index[0], dtype=np.int64)
    dst = np.asarray(edge_index[1], dtype=np.int64)

    counts = np.bincount(src, minlength=N)
    rowptr = np.zeros(N + 1, dtype=np.int64)
    np.cumsum(counts, out=rowptr[1:])

    iota = np.tile(np.tile(np.arange(128, dtype=np.float32), CPG),
                   (128, 1)).astype(BF16)
    li1 = np.tile(np.repeat(np.arange(8, dtype=np.float32), 16),
                  (128, CPG)).astype(BF16)
    li2 = np.tile(np.repeat(np.arange(4, dtype=np.float32), L2W),
                  (128, CPG)).astype(BF16)
    pmk = np.ones((128, 1), dtype=np.float32)
    ident = np.eye(128, dtype=np.float32)
    w2p = np.zeros((NHID, CPAD), dtype=np.float32)
    w2p[:, :NCLASS] = W2
    b2p = np.zeros((1, CPAD), dtype=np.float32)
    b2p[0, :NCLASS] = b2

    global _SLOT_OF
    dst_core = dst // SHARD
    # pass 1: per-core greedy slot packing (block edge cap BPC*128, 128
    # slots per block). slot_of[k][local] = slot id, -1 spare slots unused.
    slot_of = []
    node_of = []
    for k in range(NCORES):
        dk = dst[dst_core == k] - k * SHARD
        degk = np.bincount(dk, minlength=SHARD) + 1  # + self loop
        s_of = np.empty(SHARD, dtype=np.int64)
        blk, used_s, used_e = 0, 0, 0
        cap = BPC * 128
        for local in range(SHARD):
            d = degk[local]
            if used_s == 128 or used_e + d > cap:
                blk += 1
                used_s, used_e = 0, 0
            s_of[local] = blk * 128 + used_s
            used_s += 1
            used_e += d
        assert blk < NBLK, f"slot packing overflow: {blk}"
        n_of = np.full(SHARD_PAD, -1, dtype=np.int64)
        n_of[s_of] = np.arange(SHARD)
        slot_of.append(s_of)
        node_of.append(n_of)
    _SLOT_OF = slot_of

    in_maps = []
    for k in range(NCORES):
        ids = np.arange(SHARD, dtype=np.int64) + k * SHARD
        s_of = slot_of[k]
        n_of = node_of[k]
        real = n_of >= 0
        xT = np.zeros((NFEAT, SHARD_PAD), dtype=np.float32)
        xT[:, real] = x[ids][n_of[real]].T
        xT = xT.astype(BF16)

        rpl = np.zeros(SHARD_PAD, dtype=np.float32)
        rph = np.zeros(SHARD_PAD, dtype=np.float32)
        rpl[real] = rowptr[ids][n_of[real]]
        rph[real] = rowptr[ids + 1][n_of[real]]
        rp0 = np.ascontiguousarray(rpl.reshape(NBLK, 128).T)
        rp1 = np.ascontiguousarray(rph.reshape(NBLK, 128).T)

        m = dst_core == k
        es = np.concatenate([src[m], ids])
        ed_n = np.concatenate([dst[m] - k * SHARD, ids - k * SHARD])
        ed = s_of[ed_n]
        order = np.argsort(ed, kind="stable")
        es, ed = es[order], ed[order]

        blocks = ed >> 7
        bcnt = np.bincount(blocks, minlength=NBLK)
        assert bcnt.max() <= BPC * 128, f"block overflow: {bcnt.max()}"
        bstart = np.zeros(NBLK, dtype=np.int64)
        np.cumsum(bcnt[:-1], out=bstart[1:])
        pos = np.arange(len(es)) - bstart[blocks]
        slot_c = BPC * blocks + (pos >> 7)
        slot_p = pos & 127

        e_core = es // SHARD
        e_slot = np.empty(len(es), dtype=np.int64)
        for kk in range(NCORES):
            mm = e_core == kk
            e_slot[mm] = slot_of[kk][es[mm] - kk * SHARD]
        row = e_core * SHARD_PAD + e_slot
        pack1 = np.zeros((128, NCHUNK_G), dtype=np.int64)
        pack2 = np.zeros((128, NCHUNK_G), dtype=np.int64)
        lane1v = np.full((128, NCHUNK_G), 255.0, dtype=np.float32)
        lane2v = np.full((128, NCHUNK_G), 255.0, dtype=np.float32)
        dstlv = np.full((128, NCHUNK_G), 255.0, dtype=np.float32)
        pack1[slot_p, slot_c] = row >> 3
        pack2[slot_p, slot_c] = row >> 2
        lane1v[slot_p, slot_c] = (row & 7).astype(np.float32)
        lane2v[slot_p, slot_c] = (row & 3).astype(np.float32)
        dstlv[slot_p, slot_c] = (ed & 127).astype(np.float32)

        def wrap(pack):
            gx = np.empty((128, NGI * WCOL), dtype=np.int16)
            for gi in range(NGI):
                blk = pack[:, gi * CPG:(gi + 1) * CPG]   # [128 p, CPG c]
                idx_list = blk.T.ravel()                 # q = c*128 + p
                wrapped = idx_list.reshape(WCOL, 16).T   # [16, WCOL]
                gx[:, gi * WCOL:(gi + 1) * WCOL] = np.tile(
                    wrapped, (8, 1)).astype(np.int16)
            return gx

        in_maps.append({
            "xT": xT, "rp0": rp0, "rp1": rp1,
            "gidx1": wrap(pack1), "gidx2": wrap(pack2),
            "lane1": lane1v.astype(BF16), "lane2": lane2v.astype(BF16),
            "dstl": dstlv.astype(BF16),
            "liota1": li1, "liota2": li2, "iota": iota,
            "w1": np.ascontiguousarray(W1).astype(BF16),
            "b1": b1.reshape(1, NHID).astype(np.float32),
            "w2": w2p, "b2": b2p,
            "ident": ident, "padmask": pmk,
        })
    return in_maps


def kernel(**inputs):
    global _PROGRAM
    x = np.asarray(inputs["x"], dtype=np.float32)
    edge_index = np.asarray(inputs["edge_index"])
    W1 = np.asarray(inputs["W1"], dtype=np.float32)
    b1 = np.asarray(inputs["b1"], dtype=np.float32)
    W2 = np.asarray(inputs["W2"], dtype=np.float32)
    b2 = np.asarray(inputs["b2"], dtype=np.float32)

    in_maps = _host_prep(x, edge_index, W1, b1, W2, b2)

    if _PROGRAM is None:
        _PROGRAM = _build_program()

    from concourse import bass_utils
    res = bass_utils.run_bass_kernel_spmd(
        _PROGRAM, in_maps, core_ids=list(range(NCORES)))
    parts = []
    for k in range(NCORES):
        full = np.asarray(res.results[k]["out"])
        parts.append(full[_SLOT_OF[k]])
    return np.concatenate(parts, axis=0).astype(np.float32)



# revision 5
# speedup vs baseline: 1.2372x; 1.2372x over previous
"""GCN 2-layer kernel for trn2 (8 NeuronCores, SPMD) — v3.

Sharding: nodes dst-sharded across 8 cores (12500 each, padded to 12544).
Each core owns the edges whose dst lands in its shard (plus self-loops),
packed into 128-edge chunks, 34 chunks per 128-dst block (static layout).
BOTH layers gather 16-wide bf16 table rows (layer 1: dinv*(x@W1+b1);
layer 2: dinv*relu(out1), with W2/b2 applied after the scatter in the
finisher — the b2 term uses a host-precomputed sum of dinv[src] per dst).
The two layers therefore share one pack/lane/index layout: one gidx, one
lane table, one dst-lane table.  Messages are fetched with dma_gather
(8-row bf16 packs, 256B elements, int16 pack indices), lane-selected
with ONE fused DVE op per chunk (scalar_tensor_tensor: (lane-iota ==
lane) * gathered), and scatter-added per dst-block with one-hot matmuls
accumulating in PSUM; finishers collapse the 8 lanes, apply dinv scales
(+ relu / W2+b2 + log_softmax) and write out.  dinv and sumdinv come
from the host; empty slots get dinv=0 so their table rows are zero.
"""
import sys
import numpy as np

if "/opt/trn_rl_repo" not in sys.path:
    sys.path.insert(0, "/opt/trn_rl_repo")

import ml_dtypes

BF16 = ml_dtypes.bfloat16

N = 100000
NCORES = 8
SHARD = 12500
SHARD_PAD = 12544          # 128 * 98
NBLK = 98                  # dst blocks of 128 per core
BPC = 34                   # chunks per block (128 edges each)
NCHUNK = NBLK * BPC        # 3332 PE chunks
NI = 1024                  # edges per dma_gather instruction
CPG = NI // 128            # 8 chunks per gather group
NGI = (NCHUNK + CPG - 1) // CPG   # 417 gather instructions per layer
NCHUNK_G = NGI * CPG       # chunk slots incl. trailing pads
NROWS = SHARD_PAD * NCORES
NFEAT = 512
NHID = 16
NCLASS = 40
CPAD = 48                  # padded class dim
WCOL = NI // 16            # gidx columns per gather instruction (64)

_PROGRAM = None
_SLOT_OF = None


def _build_program():
    import concourse.bacc as bacc
    import concourse.mybir as mybir
    import concourse.tile as tile

    f32 = mybir.dt.float32
    bf16 = mybir.dt.bfloat16
    i16 = mybir.dt.int16
    Alu = mybir.AluOpType
    Act = mybir.ActivationFunctionType
    Axis = mybir.AxisListType

    nc = bacc.Bacc("TRN2", target_bir_lowering=False, debug=False,
                   num_devices=NCORES, num_swdge_queues=4)

    xT = nc.dram_tensor("xT", [NFEAT, SHARD_PAD], bf16,
                        kind="ExternalInput")
    dinv = nc.dram_tensor("dinv", [128, NBLK], f32, kind="ExternalInput")
    sumdT = nc.dram_tensor("sumdT", [1, SHARD_PAD], f32,
                           kind="ExternalInput")
    gidx = nc.dram_tensor("gidx", [128, NGI * WCOL], i16,
                          kind="ExternalInput")
    lane = nc.dram_tensor("lane", [128, NCHUNK_G], bf16,
                          kind="ExternalInput")
    dstl = nc.dram_tensor("dstl", [128, NCHUNK_G], bf16,
                          kind="ExternalInput")
    liota = nc.dram_tensor("liota", [128, CPG * 128], bf16,
                       kind="ExternalInput")
    iota = nc.dram_tensor("iota", [128, CPG * 128], bf16,
                          kind="ExternalInput")
    w1 = nc.dram_tensor("w1", [NFEAT, NHID], bf16, kind="ExternalInput")
    b1 = nc.dram_tensor("b1", [1, NHID], f32, kind="ExternalInput")
    w2 = nc.dram_tensor("w2", [NHID, CPAD], f32, kind="ExternalInput")
    b2 = nc.dram_tensor("b2", [1, CPAD], f32, kind="ExternalInput")
    ident = nc.dram_tensor("ident", [128, 128], f32, kind="ExternalInput")
    out = nc.dram_tensor("out", [SHARD_PAD, NCLASS], f32,
                         kind="ExternalOutput")

    with tile.TileContext(nc) as tc:
        with (
            tc.tile_pool(name="const", bufs=1) as constp,
            tc.tile_pool(name="stream", bufs=4) as streamp,
            tc.tile_pool(name="gat", bufs=6) as gatp,
            tc.tile_pool(name="work", bufs=4) as workp,
            tc.tile_pool(name="fin", bufs=3) as finp,
            tc.tile_pool(name="psum", bufs=2, space="PSUM") as psump,
            tc.tile_pool(name="dram", bufs=1, space="DRAM") as dramp,
        ):
            # ---- constants ----
            iota_t = constp.tile([128, CPG * 128], bf16)
            nc.sync.dma_start(iota_t[:], iota[:])
            li_t = constp.tile([128, CPG * 128], bf16)
            nc.sync.dma_start(li_t[:], liota[:])
            lane_t = constp.tile([128, NCHUNK_G], bf16)
            nc.sync.dma_start(lane_t[:], lane[:])
            dstl_t = constp.tile([128, NCHUNK_G], bf16)
            nc.sync.dma_start(dstl_t[:], dstl[:])
            ident_t = constp.tile([128, 128], f32)
            nc.sync.dma_start(ident_t[:], ident[:])
            w1_t = constp.tile([128, NFEAT // 128, NHID], bf16)
            nc.sync.dma_start(w1_t[:],
                              w1[:].rearrange("(a k) h -> k a h", k=128))
            b1_t = constp.tile([1, NHID], f32)
            nc.sync.dma_start(b1_t[:], b1[:])
            b1b_t = constp.tile([1, NHID], bf16)
            nc.vector.tensor_copy(out=b1b_t[:], in_=b1_t[:])
            w2_t = constp.tile([NHID, CPAD], f32)
            nc.sync.dma_start(w2_t[:], w2[:])
            b2_t = constp.tile([1, CPAD], f32)
            nc.sync.dma_start(b2_t[:], b2[:])
            dinv_t = constp.tile([128, NBLK], f32)
            nc.sync.dma_start(dinv_t[:], dinv[:])
            sumd_t = constp.tile([1, SHARD_PAD], f32)
            nc.sync.dma_start(sumd_t[:], sumdT[:])
            ones_b = constp.tile([1, 128], bf16)
            nc.vector.memset(ones_b[:], 1.0)

            def dcol(b):
                return dinv_t[:, b:b + 1]

            # ---- DRAM tables (16-wide bf16 rows, packs of 8) ----
            t1shard = dramp.tile([SHARD_PAD, NHID], bf16)
            t1full = dramp.tile([NROWS, NHID], bf16)
            t2shard = dramp.tile([SHARD_PAD, NHID], bf16)
            t2full = dramp.tile([NROWS, NHID], bf16)
            t1packs = t1full[:].rearrange("(q r) h -> q (r h)", r=8)
            t2packs = t2full[:].rearrange("(q r) h -> q (r h)", r=8)

            # ---- phase 1: t1 rows = dinv * (x @ W1 + b1) ----
            for c in range(NBLK):
                ph = psump.tile([128, NHID], f32, tag="ph1")
                xk = streamp.tile([128, NFEAT // 128, 128], bf16, tag="xk")
                nc.sync.dma_start(
                    xk[:],
                    xT[:, c * 128:(c + 1) * 128].rearrange(
                        "(k p) c -> p k c", p=128))
                for kk in range(NFEAT // 128):
                    nc.tensor.matmul(out=ph[:], lhsT=xk[:, kk, :],
                                     rhs=w1_t[:, kk, :],
                                     start=(kk == 0), stop=False)
                nc.tensor.matmul(out=ph[:], lhsT=ones_b[:], rhs=b1b_t[:],
                                 start=False, stop=True)
                hs = workp.tile([128, NHID], bf16, tag="hs1")
                nc.scalar.activation(out=hs[:], in_=ph[:], func=Act.Copy,
                                     scale=dcol(c))
                nc.sync.dma_start(t1shard[c * 128:(c + 1) * 128, :], hs[:])

            nc.gpsimd.collective_compute(
                "AllGather", Alu.bypass,
                ins=[t1shard[:].opt()], outs=[t1full[:].opt()],
                replica_groups=[list(range(NCORES))],
            )

            # ---- finishers ----
            def collapse(acc, tagsfx):
                # acc: [128, 128] psum f32, cols = 16*lane + feat
                cp = finp.tile([128, 128], f32, tag="cp" + tagsfx)
                nc.scalar.activation(out=cp[:], in_=acc[:, 0:128],
                                     func=Act.Copy)
                s1 = finp.tile([128, 64], f32, tag="s1" + tagsfx)
                nc.vector.tensor_tensor(out=s1[:], in0=cp[:, 0:64],
                                        in1=cp[:, 64:128], op=Alu.add)
                s2 = finp.tile([128, 32], f32, tag="s2" + tagsfx)
                nc.vector.tensor_tensor(out=s2[:], in0=s1[:, 0:32],
                                        in1=s1[:, 32:64], op=Alu.add)
                s3 = finp.tile([128, NHID], f32, tag="s3" + tagsfx)
                nc.vector.tensor_tensor(out=s3[:], in0=s2[:, 0:16],
                                        in1=s2[:, 16:32], op=Alu.add)
                return s3

            def finish_block1(b, acc):
                s3 = collapse(acc, "a")
                r = finp.tile([128, NHID], f32, tag="r1")
                nc.scalar.activation(out=r[:], in_=s3[:], func=Act.Relu,
                                     scale=dcol(b))
                t2row = finp.tile([128, NHID], bf16, tag="t2row")
                nc.scalar.activation(out=t2row[:], in_=r[:],
                                     func=Act.Copy, scale=dcol(b))
                nc.sync.dma_start(t2shard[b * 128:(b + 1) * 128, :],
                                  t2row[:])

            def finish_block2(b, acc):
                s3 = collapse(acc, "b")
                pt = psump.tile([NHID, 128], f32, tag="ptr")
                nc.tensor.transpose(out=pt[:], in_=s3[:],
                                    identity=ident_t[:])
                a2T = finp.tile([NHID, 128], f32, tag="a2T")
                nc.vector.tensor_copy(out=a2T[:], in_=pt[:])
                ph2 = psump.tile([128, CPAD], f32, tag="ph2")
                nc.tensor.matmul(out=ph2[:], lhsT=a2T[:], rhs=w2_t[:],
                                 start=True, stop=False)
                nc.tensor.matmul(out=ph2[:],
                                 lhsT=sumd_t[:, b * 128:(b + 1) * 128],
                                 rhs=b2_t[:], start=False, stop=True)
                o2 = finp.tile([128, NCLASS], f32, tag="o2")
                nc.scalar.activation(out=o2[:], in_=ph2[:, 0:NCLASS],
                                     func=Act.Copy, scale=dcol(b))
                rmax = finp.tile([128, 1], f32, tag="rmax")
                nc.vector.tensor_reduce(out=rmax[:], in_=o2[:],
                                        axis=Axis.X, op=Alu.max)
                nrmax = finp.tile([128, 1], f32, tag="nrmax")
                nc.vector.tensor_scalar(out=nrmax[:], in0=rmax[:],
                                        scalar1=-1.0, scalar2=None,
                                        op0=Alu.mult)
                sh = finp.tile([128, NCLASS], f32, tag="sh")
                nc.scalar.activation(out=sh[:], in_=o2[:],
                                     func=Act.Identity, bias=nrmax[:])
                ex = finp.tile([128, NCLASS], f32, tag="ex")
                nc.scalar.activation(out=ex[:], in_=sh[:], func=Act.Exp)
                rsum = finp.tile([128, 1], f32, tag="rsum")
                nc.vector.tensor_reduce(out=rsum[:], in_=ex[:],
                                        axis=Axis.X, op=Alu.add)
                rinv = finp.tile([128, 1], f32, tag="rinv")
                nc.vector.reciprocal(out=rinv[:], in_=rsum[:])
                nlsum = finp.tile([128, 1], f32, tag="nlsum")
                nc.scalar.activation(out=nlsum[:], in_=rinv[:], func=Act.Ln)
                res = finp.tile([128, NCLASS], f32, tag="res")
                nc.scalar.activation(out=res[:], in_=sh[:],
                                     func=Act.Identity, bias=nlsum[:])
                nc.sync.dma_start(out[b * 128:(b + 1) * 128, :], res[:])

            # ---- message-passing phase ----
            def mp_phase(packs_ap, finish):
                acc_holder = [None]
                for gi in range(NGI):
                    cs = slice(gi * CPG, gi * CPG + CPG)
                    gx = gatp.tile([128, WCOL], i16, tag="gx", bufs=16)
                    nc.sync.dma_start(gx[:],
                                      gidx[:, gi * WCOL:(gi + 1) * WCOL])
                    gat = gatp.tile([128, CPG, 128], bf16, tag="gt",
                                    bufs=10)
                    nc.gpsimd.dma_gather(gat[:], packs_ap, gx[:], NI, NI,
                                         128, queue_num=gi % 4)
                    onehot = gatp.tile([128, CPG, 128], bf16, tag="oh",
                                       bufs=10)
                    nc.vector.tensor_tensor(
                        out=onehot[:],
                        in0=dstl_t[:, cs].to_broadcast([128, CPG, 128]),
                        in1=iota_t[:].rearrange("p (c e) -> p c e", e=128),
                        op=Alu.is_equal)
                    msk = workp.tile([128, CPG, 128], bf16,
                                     tag="mk", bufs=6)
                    nc.vector.tensor_tensor(
                        out=msk[:],
                        in0=lane_t[:, cs].to_broadcast([128, CPG, 128]),
                        in1=li_t[:].rearrange("p (c e) -> p c e", e=128),
                        op=Alu.is_equal)
                    gmall = workp.tile([128, CPG, 128], bf16,
                                       tag="gm", bufs=6)
                    nc.vector.tensor_tensor(out=gmall[:], in0=gat[:],
                                            in1=msk[:], op=Alu.mult)
                    for j in range(CPG):
                        c = gi * CPG + j
                        if c >= NCHUNK:
                            continue
                        b, jj = divmod(c, BPC)
                        if jj == 0:
                            acc_holder[0] = psump.tile(
                                [128, 128], f32, tag="acc", name="acc_t")
                        nc.tensor.matmul(
                            out=acc_holder[0][:],
                            lhsT=onehot[:, j, :], rhs=gmall[:, j, :],
                            start=(jj == 0), stop=(jj == BPC - 1))
                        if jj == BPC - 1:
                            finish(b, acc_holder[0])

            # ---- phase 2: layer 1 edges ----
            mp_phase(t1packs, finish_block1)

            nc.gpsimd.collective_compute(
                "AllGather", Alu.bypass,
                ins=[t2shard[:].opt()], outs=[t2full[:].opt()],
                replica_groups=[list(range(NCORES))],
            )

            # ---- phase 4: layer 2 edges ----
            mp_phase(t2packs, finish_block2)

    nc.compile()
    return nc


def _host_prep(x, edge_index, W1, b1, W2, b2):
    src = np.asarray(edge_index[0], dtype=np.int64)
    dst = np.asarray(edge_index[1], dtype=np.int64)

    outdeg = np.bincount(src, minlength=N).astype(np.float64) + 1.0
    dinv_g = 1.0 / np.sqrt(outdeg)           # float64

    iota = np.tile(np.tile(np.arange(128, dtype=np.float32), CPG),
                   (128, 1)).astype(BF16)
    liota = np.tile(np.repeat(np.arange(8, dtype=np.float32), 16),
                    (128, CPG)).astype(BF16)
    ident = np.eye(128, dtype=np.float32)
    w2p = np.zeros((NHID, CPAD), dtype=np.float32)
    w2p[:, :NCLASS] = W2
    b2p = np.zeros((1, CPAD), dtype=np.float32)
    b2p[0, :NCLASS] = b2

    global _SLOT_OF
    dst_core = dst // SHARD
    # pass 1: per-core greedy slot packing (block edge cap BPC*128, 128
    # slots per block).
    slot_of = []
    node_of = []
    for k in range(NCORES):
        dk = dst[dst_core == k] - k * SHARD
        degk = np.bincount(dk, minlength=SHARD) + 1  # + self loop
        s_of = np.empty(SHARD, dtype=np.int64)
        blk, used_s, used_e = 0, 0, 0
        cap = BPC * 128
        for local in range(SHARD):
            d = degk[local]
            if used_s == 128 or used_e + d > cap:
                blk += 1
                used_s, used_e = 0, 0
            s_of[local] = blk * 128 + used_s
            used_s += 1
            used_e += d
        assert blk < NBLK, f"slot packing overflow: {blk}"
        n_of = np.full(SHARD_PAD, -1, dtype=np.int64)
        n_of[s_of] = np.arange(SHARD)
        slot_of.append(s_of)
        node_of.append(n_of)
    _SLOT_OF = slot_of

    in_maps = []
    for k in range(NCORES):
        ids = np.arange(SHARD, dtype=np.int64) + k * SHARD
        s_of = slot_of[k]
        n_of = node_of[k]
        real = n_of >= 0
        xT = np.zeros((NFEAT, SHARD_PAD), dtype=np.float32)
        xT[:, real] = x[ids][n_of[real]].T
        xT = xT.astype(BF16)

        dinv_slot = np.zeros(SHARD_PAD, dtype=np.float32)
        dinv_slot[real] = dinv_g[ids][n_of[real]].astype(np.float32)
        dinv_in = np.ascontiguousarray(dinv_slot.reshape(NBLK, 128).T)

        m = dst_core == k
        es = np.concatenate([src[m], ids])
        ed_n = np.concatenate([dst[m] - k * SHARD, ids - k * SHARD])
        ed = s_of[ed_n]
        order = np.argsort(ed, kind="stable")
        es, ed = es[order], ed[order]

        sumd = np.zeros(SHARD_PAD, dtype=np.float64)
        np.add.at(sumd, ed, dinv_g[es])
        sumdT = sumd.astype(np.float32).reshape(1, SHARD_PAD)

        blocks = ed >> 7
        bcnt = np.bincount(blocks, minlength=NBLK)
        assert bcnt.max() <= BPC * 128, f"block overflow: {bcnt.max()}"
        bstart = np.zeros(NBLK, dtype=np.int64)
        np.cumsum(bcnt[:-1], out=bstart[1:])
        pos = np.arange(len(es)) - bstart[blocks]
        slot_c = BPC * blocks + (pos >> 7)
        slot_p = pos & 127

        e_core = es // SHARD
        e_slot = np.empty(len(es), dtype=np.int64)
        for kk in range(NCORES):
            mm = e_core == kk
            e_slot[mm] = slot_of[kk][es[mm] - kk * SHARD]
        row = e_core * SHARD_PAD + e_slot
        pack = np.zeros((128, NCHUNK_G), dtype=np.int64)
        lanev = np.full((128, NCHUNK_G), 255.0, dtype=np.float32)
        dstlv = np.full((128, NCHUNK_G), 255.0, dtype=np.float32)
        pack[slot_p, slot_c] = row >> 3
        lanev[slot_p, slot_c] = (row & 7).astype(np.float32)
        dstlv[slot_p, slot_c] = (ed & 127).astype(np.float32)

        gx = np.empty((128, NGI * WCOL), dtype=np.int16)
        for gi in range(NGI):
            blk = pack[:, gi * CPG:(gi + 1) * CPG]   # [128 p, CPG c]
            idx_list = blk.T.ravel()                 # q = c*128 + p
            wrapped = idx_list.reshape(WCOL, 16).T   # [16, WCOL]
            gx[:, gi * WCOL:(gi + 1) * WCOL] = np.tile(
                wrapped, (8, 1)).astype(np.int16)

        in_maps.append({
            "xT": xT, "dinv": dinv_in, "sumdT": sumdT,
            "gidx": gx,
            "lane": lanev.astype(BF16),
            "dstl": dstlv.astype(BF16),
            "liota": liota, "iota": iota,
            "w1": np.ascontiguousarray(W1).astype(BF16),
            "b1": b1.reshape(1, NHID).astype(np.float32),
            "w2": w2p, "b2": b2p,
            "ident": ident,
        })
    return in_maps


def kernel(**inputs):
    global _PROGRAM
    x = np.asarray(inputs["x"], dtype=np.float32)
    edge_index = np.asarray(inputs["edge_index"])
    W1 = np.asarray(inputs["W1"], dtype=np.float32)
    b1 = np.asarray(inputs["b1"], dtype=np.float32)
    W2 = np.asarray(inputs["W2"], dtype=np.float32)
    b2 = np.asarray(inputs["b2"], dtype=np.float32)

    in_maps = _host_prep(x, edge_index, W1, b1, W2, b2)

    if _PROGRAM is None:
        _PROGRAM = _build_program()

    from concourse import bass_utils
    res = bass_utils.run_bass_kernel_spmd(
        _PROGRAM, in_maps, core_ids=list(range(NCORES)))
    parts = []
    for k in range(NCORES):
        full = np.asarray(res.results[k]["out"])
        parts.append(full[_SLOT_OF[k]])
    return np.concatenate(parts, axis=0).astype(np.float32)
